# revision 7
# baseline (speedup 1.0000x reference)
"""Channel attention (B=2, N=8192, C=64) on 8 Trainium2 NeuronCores.

Math per batch b:  q = x[b] reshaped (N, C)
    energy = q @ q.T              (N, N)
    attn   = softmax(energy, -1)
    out    = gamma * (attn @ q) + x[b]

Sharding: core = (b, j) handles queries rows j*2048:(j+1)*2048 of batch b.
Each core receives the full x[b] in two layouts, ROLLED so its own query
range sits at rows 0:2048 (makes the SPMD program offset-free):
    xt (C=64, N=8192)   : x[b].T  (C on partitions)  -> S^T matmul operands
    xn (N=8192, C=64)   : x[b]    natural            -> PV matmul lhsT + residual

On-device algorithm (keys-on-partitions "S^T" orientation, zero transposes
in the main loop):
    S^T tile = matmul(lhsT=[xt_k ; ones] (65,128), rhs=[xt_q ; -m_q] (65,512))
      -> (128 keys, 512 queries) scores pre-shifted by -m_q = -||x_q||^2
         (the diagonal IS the row max for this operator: energy[n,n] = ||q_n||^2
         dominates every off-diagonal dot product, so exp args are <= 0)
    P^T = exp(S^T)  on ScalarE (PSUM -> SBUF)
    O'  += matmul(lhsT=[xn_k | ones] (128,65), rhs=P^T (128,512))
      -> rows 0..63 accumulate O^T = (attn-unnormalized)^T, row 64 accumulates
         the softmax denominator. Accumulated in PSUM over all 64 key chunks.
    Epilogue: PE-transpose O' 128-query blocks to natural layout, multiply by
    1/denom * gamma, add residual, DMA out.
"""

from contextlib import ExitStack

import numpy as np

import concourse.bass as bass
import concourse.mybir as mybir
import concourse.tile as tile
from concourse.bass_utils import run_bass_kernel_spmd
from concourse.masks import make_identity

B, D, H, W, C = 2, 8, 32, 32, 64
N = D * H * W          # 8192
NCORES = 8
QPC = (B * N) // NCORES  # 2048 queries per core
KC = 128               # key-chunk size (S^T tile partition dim)
NKC = N // KC          # 64
QT = 1024              # query tile (half of QPC) -> psum S^T buf (128, 1024)
NQH = QPC // QT        # 2
MMF = 512              # moving free dim per f32 matmul
F32 = mybir.dt.float32
AF = mybir.ActivationFunctionType
ALU = mybir.AluOpType


_SPLIT_WAIT_TYPES = (
    "InstMatmult", "InstActivation", "InstTensorTensor", "InstTensorScalarPtr",
    "InstTensorScalarAffineSelect", "InstTensorReduce", "InstTensorCopy",
    "InstReciprocal", "InstMemset", "InstIota", "InstCopy",
    "InstTensorTensorScan", "InstStreamTranspose", "InstCopyPredicated",
    "InstDMACopy", "InstDrain", "InstEventSemaphore",
)


def _split_matmul_waits(nc: bass.Bass) -> None:
    """This walrus build allows only ONE sync wait per compute-engine
    instruction (e.g. Matmult's LDWEIGHTS micro-inst and Activation's ISA
    struct each have a single wait slot).  Tile's sem assigner doesn't know
    that, so move all but one wait onto single-wait NoOps inserted right
    before the instruction in its basic block (= right before it in that
    engine's stream)."""
    n_fix = 0
    for f in nc.m.functions:
        for bb in f.blocks:
            il = bb.instructions
            out = []
            changed = False
            for inst in il:
                si = inst.sync_info
                if (
                    type(inst).__name__ in _SPLIT_WAIT_TYPES
                    and si is not None
                    and len(si.on_wait) > 1
                ):
                    waits = list(si.on_wait)
                    for w_i, w in enumerate(waits[:-1]):
                        nop = mybir.InstEventSemaphore(
                            name=f"{inst.name}-wn{w_i}", engine=inst.engine,
                            ins=[], outs=[],
                        )
                        nop.sync_info = mybir.SyncInfo(on_wait=[w], on_update=[])
                        out.append(nop)
                    inst.sync_info = mybir.SyncInfo(
                        on_wait=[waits[-1]], on_update=list(si.on_update)
                    )
                    changed = True
                    n_fix += 1
                out.append(inst)
            if changed:
                bb.instructions = out
    if n_fix:
        print(f"kernel: split waits on {n_fix} matmuls")


def _build() -> bass.Bass:
    nc = bass.Bass()
    xt_d = nc.declare_dram_parameter("xt", [C, N], F32, isOutput=False)
    xn_d = nc.declare_dram_parameter("xn", [N, C], F32, isOutput=False)
    gamma_d = nc.declare_dram_parameter("gamma", [1, 1], F32, isOutput=False)
    out_d = nc.declare_dram_parameter("out", [QPC, C], F32, isOutput=True)

    with ExitStack() as ctx:
        tc = ctx.enter_context(tile.TileContext(nc))
        const = ctx.enter_context(tc.tile_pool(name="const", bufs=1))
        big = ctx.enter_context(tc.tile_pool(name="big", bufs=1))
        ptp = ctx.enter_context(tc.tile_pool(name="ptp", bufs=2))
        work = ctx.enter_context(tc.tile_pool(name="work", bufs=2))
        outp = ctx.enter_context(tc.tile_pool(name="outp", bufs=3))
        ps_s = ctx.enter_context(tc.tile_pool(name="ps_s", bufs=2, space="PSUM"))
        ps_o = ctx.enter_context(tc.tile_pool(name="ps_o", bufs=1, space="PSUM"))
        ps_t = ctx.enter_context(tc.tile_pool(name="ps_t", bufs=2, space="PSUM"))

        # ---- constants ----
        ident = const.tile([C + 1, C + 1], F32)
        make_identity(nc, ident)
        ones_col = const.tile([C, 1], F32)
        nc.vector.memset(ones_col, 1.0)
        gam = const.tile([128, 1], F32)
        g_ap = gamma_d[:, :]
        nc.sync.dma_start(
            out=gam,
            in_=bass.AP(tensor=g_ap.tensor, offset=g_ap.offset, ap=[[0, 128], [1, 1]]),
        )

        # ---- x^T with a ones row appended: (65, 8192) ----
        xt1 = big.tile([C + 1, N], F32)
        nc.sync.dma_start(out=xt1[0:C, :], in_=xt_d[:, :])
        nc.gpsimd.memset(xt1[C : C + 1, :], 1.0)

        # ---- x natural, chunked (128, 64, 65): col 64 of each chunk = ones ----
        xna = big.tile([128, NKC * (C + 1)], F32)
        xna_v = xna.rearrange("p (k c) -> p k c", c=C + 1)
        nc.sync.dma_start(
            out=xna_v[:, :, 0:C],
            in_=xn_d[:, :].rearrange("(k p) c -> p k c", p=128),
        )
        nc.gpsimd.memset(xna_v[:, :, C : C + 1], 1.0)

        # ---- rhs_aug (65, 2048): rows 0..63 = xt[:, 0:QPC], row 64 = -||x_q||^2
        rhs = big.tile([C + 1, QPC], F32)
        nc.sync.dma_start(out=rhs[0:C, :], in_=xt_d[:, 0:QPC])
        sq = big.tile([C, QPC], F32)
        nc.scalar.square(sq, rhs[0:C, :])
        for i in range(QPC // MMF):
            pm = ps_s.tile([1, MMF], F32, tag="s")
            nc.tensor.matmul(
                pm, lhsT=ones_col, rhs=sq[:, i * MMF : (i + 1) * MMF],
                start=True, stop=True,
            )
            nc.scalar.mul(rhs[C : C + 1, i * MMF : (i + 1) * MMF], pm, -1.0)

        # ---- main loop ----
        for qh in range(NQH):
            po = ps_o.tile([C + 1, QT], F32, tag="o")
            for k in range(NKC):
                ps = ps_s.tile([128, QT], F32, tag="s")
                for i in range(QT // MMF):
                    nc.tensor.matmul(
                        ps[:, i * MMF : (i + 1) * MMF],
                        lhsT=xt1[:, k * KC : (k + 1) * KC],
                        rhs=rhs[:, qh * QT + i * MMF : qh * QT + (i + 1) * MMF],
                        start=True, stop=True,
                    )
                pt = ptp.tile([128, QT], F32, tag="pt")
                nc.scalar.activation(pt, ps, AF.Exp)
                for i in range(QT // MMF):
                    nc.tensor.matmul(
                        po[:, i * MMF : (i + 1) * MMF],
                        lhsT=xna_v[:, k, :],
                        rhs=pt[:, i * MMF : (i + 1) * MMF],
                        start=(k == 0), stop=(k == NKC - 1),
                    )
            # epilogue: normalize, scale, residual, store
            oc = work.tile([C + 1, QT], F32, tag="oc")
            nc.scalar.copy(oc, po)
            for blk in range(QT // 128):
                ptr = ps_t.tile([128, C + 1], F32, tag="t")
                nc.tensor.transpose(ptr, oc[:, blk * 128 : (blk + 1) * 128], ident)
                rd = outp.tile([128, 1], F32, tag="rd")
                nc.vector.reciprocal(rd, ptr[:, C : C + 1])
                rdg = outp.tile([128, 1], F32, tag="rdg")
                nc.vector.tensor_tensor(rdg, rd, gam, op=ALU.mult)
                ob = outp.tile([128, C], F32, tag="ob")
                nc.vector.scalar_tensor_tensor(
                    out=ob,
                    in0=ptr[:, 0:C],
                    scalar=rdg,
                    in1=xna_v[:, qh * (QT // 128) + blk, 0:C],
                    op0=ALU.mult,
                    op1=ALU.add,
                )
                nc.sync.dma_start(
                    out=out_d[:, :].rearrange("(t p) c -> t p c", p=128)[
                        qh * (QT // 128) + blk
                    ],
                    in_=ob,
                )
    _split_matmul_waits(nc)
    return nc


_PROG: bass.Bass | None = None


def _get_prog() -> bass.Bass:
    global _PROG
    if _PROG is None:
        _PROG = _build()
    return _PROG


def kernel(x: np.ndarray, gamma: np.ndarray) -> np.ndarray:
    x = np.ascontiguousarray(np.asarray(x, dtype=np.float32))
    g = np.ascontiguousarray(np.asarray(gamma, dtype=np.float32)).reshape(1, 1)
    xf = x.reshape(B, N, C)
    per_b = NCORES // B
    in_maps = []
    for core in range(NCORES):
        b, j = divmod(core, per_b)
        xr = np.roll(xf[b], -j * QPC, axis=0)
        in_maps.append(
            {
                "xt": np.ascontiguousarray(xr.T),
                "xn": np.ascontiguousarray(xr),
                "gamma": g,
            }
        )
    res = run_bass_kernel_spmd(_get_prog(), in_maps, list(range(NCORES))).results
    out = np.empty((B, N, C), dtype=np.float32)
    for core in range(NCORES):
        b, j = divmod(core, per_b)
        out[b, j * QPC : (j + 1) * QPC] = res[core]["out"]
    return out.reshape(B, D, H, W, C)


if __name__ == "__main__":
    _build()
    print("build ok")


# revision 13
# speedup vs baseline: 2.4301x; 2.4301x over previous
"""Channel attention (B=2, N=8192, C=64) on 8 Trainium2 NeuronCores.

Math per batch b:  q = x[b] reshaped (N, C)
    energy = q @ q.T              (N, N)
    attn   = softmax(energy, -1)
    out    = gamma * (attn @ q) + x[b]

Sharding: core = (b, j) handles queries rows j*2048:(j+1)*2048 of batch b.
Each core receives the full x[b] in two layouts, ROLLED so its own query
range sits at rows 0:2048 (makes the SPMD program offset-free):
    xt (C=64, N=8192)   : x[b].T  (C on partitions)  -> S^T matmul operands
    xn (N=8192, C=64)   : x[b]    natural            -> PV matmul lhsT + residual

On-device algorithm (keys-on-partitions "S^T" orientation, zero transposes
in the main loop):
    S^T tile = matmul(lhsT=[xt_k ; ones] (65,128), rhs=[xt_q ; -m_q] (65,512))
      -> (128 keys, 512 queries) scores pre-shifted by -m_q = -||x_q||^2
         (the diagonal IS the row max for this operator: energy[n,n] = ||q_n||^2
         dominates every off-diagonal dot product, so exp args are <= 0)
    P^T = exp(S^T)  on ScalarE (PSUM -> SBUF)
    O'  += matmul(lhsT=[xn_k | ones] (128,65), rhs=P^T (128,512))
      -> rows 0..63 accumulate O^T = (attn-unnormalized)^T, row 64 accumulates
         the softmax denominator. Accumulated in PSUM over all 64 key chunks.
    Epilogue: PE-transpose O' 128-query blocks to natural layout, multiply by
    1/denom * gamma, add residual, DMA out.
"""

from contextlib import ExitStack

import numpy as np

import concourse.bass as bass
import concourse.mybir as mybir
import concourse.tile as tile
from concourse.bass_utils import run_bass_kernel_spmd
from concourse.masks import make_identity

B, D, H, W, C = 2, 8, 32, 32, 64
N = D * H * W          # 8192
NCORES = 8
QPC = (B * N) // NCORES  # 2048 queries per core
KC = 128               # key-chunk size (S^T tile partition dim)
NKC = N // KC          # 64
QT = 1024              # query tile (half of QPC) -> psum S^T buf (128, 1024)
NQH = QPC // QT        # 2
MMF = 512              # moving free dim per f32 matmul
F32 = mybir.dt.float32
F32R = mybir.dt.float32r  # fp32 bits, 1 cycle/row matmul when N >= 256 (vs 4 for f32)
AF = mybir.ActivationFunctionType
ALU = mybir.AluOpType


def _r(ap):
    return ap.bitcast(F32R)


def _f(ap):
    return ap.bitcast(F32)


_SPLIT_WAIT_TYPES = (
    "InstMatmult", "InstActivation", "InstTensorTensor", "InstTensorScalarPtr",
    "InstTensorScalarAffineSelect", "InstTensorReduce", "InstTensorCopy",
    "InstReciprocal", "InstMemset", "InstIota", "InstCopy",
    "InstTensorTensorScan", "InstStreamTranspose", "InstCopyPredicated",
    "InstDMACopy", "InstDrain", "InstEventSemaphore",
)


def _split_matmul_waits(nc: bass.Bass) -> None:
    """This walrus build allows only ONE sync wait per compute-engine
    instruction (e.g. Matmult's LDWEIGHTS micro-inst and Activation's ISA
    struct each have a single wait slot).  Tile's sem assigner doesn't know
    that, so move all but one wait onto single-wait NoOps inserted right
    before the instruction in its basic block (= right before it in that
    engine's stream)."""
    n_fix = 0
    for f in nc.m.functions:
        for bb in f.blocks:
            il = bb.instructions
            out = []
            changed = False
            for inst in il:
                si = inst.sync_info
                if (
                    type(inst).__name__ in _SPLIT_WAIT_TYPES
                    and si is not None
                    and len(si.on_wait) > 1
                ):
                    waits = list(si.on_wait)
                    for w_i, w in enumerate(waits[:-1]):
                        nop = mybir.InstEventSemaphore(
                            name=f"{inst.name}-wn{w_i}", engine=inst.engine,
                            ins=[], outs=[],
                        )
                        nop.sync_info = mybir.SyncInfo(on_wait=[w], on_update=[])
                        out.append(nop)
                    inst.sync_info = mybir.SyncInfo(
                        on_wait=[waits[-1]], on_update=list(si.on_update)
                    )
                    changed = True
                    n_fix += 1
                out.append(inst)
            if changed:
                bb.instructions = out
    if n_fix:
        print(f"kernel: split waits on {n_fix} matmuls")


def _build() -> bass.Bass:
    nc = bass.Bass()
    xt_d = nc.declare_dram_parameter("xt", [C, N], F32, isOutput=False)
    xn_d = nc.declare_dram_parameter("xn", [N, C], F32, isOutput=False)
    gamma_d = nc.declare_dram_parameter("gamma", [1, 1], F32, isOutput=False)
    out_d = nc.declare_dram_parameter("out", [QPC, C], F32, isOutput=True)

    with ExitStack() as ctx:
        tc = ctx.enter_context(tile.TileContext(nc))
        const = ctx.enter_context(tc.tile_pool(name="const", bufs=1))
        big = ctx.enter_context(tc.tile_pool(name="big", bufs=1))
        ptp = ctx.enter_context(tc.tile_pool(name="ptp", bufs=2))
        work = ctx.enter_context(tc.tile_pool(name="work", bufs=2))
        outp = ctx.enter_context(tc.tile_pool(name="outp", bufs=3))
        ps_s = ctx.enter_context(tc.tile_pool(name="ps_s", bufs=2, space="PSUM"))
        ps_o = ctx.enter_context(tc.tile_pool(name="ps_o", bufs=1, space="PSUM"))
        ps_t = ctx.enter_context(tc.tile_pool(name="ps_t", bufs=2, space="PSUM"))

        # ---- constants ----
        ident = const.tile([C + 1, C + 1], F32)
        make_identity(nc, ident)
        ones_col = const.tile([C, 1], F32R)
        nc.vector.memset(_f(ones_col), 1.0)
        gam = const.tile([128, 1], F32)
        g_ap = gamma_d[:, :]
        nc.sync.dma_start(
            out=gam,
            in_=bass.AP(tensor=g_ap.tensor, offset=g_ap.offset, ap=[[0, 128], [1, 1]]),
        )

        # ---- x^T with a ones row appended: (65, 8192) ----
        xt1 = big.tile([C + 1, N], F32R)
        nc.sync.dma_start(out=xt1[0:C, :], in_=xt_d[:, :].bitcast(F32R))
        nc.gpsimd.memset(_f(xt1[C : C + 1, :]), 1.0)

        # ---- x natural, chunked (128, 64, 65): col 64 of each chunk = ones ----
        xna = big.tile([128, NKC * (C + 1)], F32R)
        xna_v = xna.rearrange("p (k c) -> p k c", c=C + 1)
        nc.sync.dma_start(
            out=xna_v[:, :, 0:C],
            in_=xn_d[:, :].rearrange("(k p) c -> p k c", p=128).bitcast(F32R),
        )
        nc.gpsimd.memset(_f(xna_v[:, :, C : C + 1]), 1.0)

        # ---- rhs_aug (65, 2048): rows 0..63 = xt[:, 0:QPC], row 64 = -||x_q||^2
        rhs = big.tile([C + 1, QPC], F32R)
        nc.sync.dma_start(out=rhs[0:C, :], in_=xt_d[:, 0:QPC].bitcast(F32R))
        sq = big.tile([C, QPC], F32R)
        nc.scalar.square(sq, rhs[0:C, :])
        for i in range(QPC // MMF):
            pm = ps_s.tile([1, MMF], F32, tag="s")
            nc.tensor.matmul(
                pm, lhsT=ones_col, rhs=sq[:, i * MMF : (i + 1) * MMF],
                start=True, stop=True,
            )
            nc.scalar.mul(rhs[C : C + 1, i * MMF : (i + 1) * MMF], pm, -1.0)

        # ---- main loop ----
        for qh in range(NQH):
            po = ps_o.tile([C + 1, QT], F32, tag="o")
            for k in range(NKC):
                ps = ps_s.tile([128, QT], F32, tag="s")
                for i in range(QT // MMF):
                    nc.tensor.matmul(
                        ps[:, i * MMF : (i + 1) * MMF],
                        lhsT=xt1[:, k * KC : (k + 1) * KC],
                        rhs=rhs[:, qh * QT + i * MMF : qh * QT + (i + 1) * MMF],
                        start=True, stop=True,
                    )
                pt = ptp.tile([128, QT], F32R, tag="pt")
                nc.scalar.activation(pt, ps, AF.Exp)
                for i in range(QT // MMF):
                    nc.tensor.matmul(
                        po[:, i * MMF : (i + 1) * MMF],
                        lhsT=xna_v[:, k, :],
                        rhs=pt[:, i * MMF : (i + 1) * MMF],
                        start=(k == 0), stop=(k == NKC - 1),
                    )
            # epilogue: normalize, scale, residual, store
            oc = work.tile([C + 1, QT], F32, tag="oc")
            nc.scalar.copy(oc, po)
            for blk in range(QT // 128):
                ptr = ps_t.tile([128, C + 1], F32, tag="t")
                nc.tensor.transpose(ptr, oc[:, blk * 128 : (blk + 1) * 128], ident)
                rd = outp.tile([128, 1], F32, tag="rd")
                nc.vector.reciprocal(rd, ptr[:, C : C + 1])
                rdg = outp.tile([128, 1], F32, tag="rdg")
                nc.vector.tensor_tensor(rdg, rd, gam, op=ALU.mult)
                ob = outp.tile([128, C], F32, tag="ob")
                nc.vector.scalar_tensor_tensor(
                    out=ob,
                    in0=ptr[:, 0:C],
                    scalar=rdg,
                    in1=_f(xna_v[:, qh * (QT // 128) + blk, 0:C]),
                    op0=ALU.mult,
                    op1=ALU.add,
                )
                nc.sync.dma_start(
                    out=out_d[:, :].rearrange("(t p) c -> t p c", p=128)[
                        qh * (QT // 128) + blk
                    ],
                    in_=ob,
                )
    _split_matmul_waits(nc)
    return nc


_PROG: bass.Bass | None = None


def _get_prog() -> bass.Bass:
    global _PROG
    if _PROG is None:
        _PROG = _build()
    return _PROG


def kernel(x: np.ndarray, gamma: np.ndarray) -> np.ndarray:
    x = np.ascontiguousarray(np.asarray(x, dtype=np.float32))
    g = np.ascontiguousarray(np.asarray(gamma, dtype=np.float32)).reshape(1, 1)
    xf = x.reshape(B, N, C)
    per_b = NCORES // B
    in_maps = []
    for core in range(NCORES):
        b, j = divmod(core, per_b)
        xr = np.roll(xf[b], -j * QPC, axis=0)
        in_maps.append(
            {
                "xt": np.ascontiguousarray(xr.T),
                "xn": np.ascontiguousarray(xr),
                "gamma": g,
            }
        )
    res = run_bass_kernel_spmd(_get_prog(), in_maps, list(range(NCORES))).results
    out = np.empty((B, N, C), dtype=np.float32)
    for core in range(NCORES):
        b, j = divmod(core, per_b)
        out[b, j * QPC : (j + 1) * QPC] = res[core]["out"]
    return out.reshape(B, D, H, W, C)


if __name__ == "__main__":
    _build()
    print("build ok")
